# revision 1
# baseline (speedup 1.0000x reference)
"""Trainium2 Bass kernel for nn_GeometricEmbedding (GNN message passing).

Strategy (8 NeuronCores, graph-partitioned queries):
  * Host: sort edges by query id, partition queries contiguously across the 8
    cores (12500/core), CSR-pad each query's neighbor list to K slots with the
    query's own position (so displacement u = nbr - qpos == 0 in pad slots and
    every segment sum needs no masking). Host work is pure indexing/gather as
    suggested by the sharding hint; all arithmetic runs on device.
  * Device, per core: u = nbr - qpos; per-query segment sums of
    [u, u*u (6 moments), ||u||, ||u||^2] via free-axis reductions; closed-form
    3x3 symmetric eigensolve (trig method via Arctan/Sin activations);
    feature standardization via a [9,2] AllReduce folded into W1/b1;
    2-layer MLP on the tensor engine; output written directly per core.
"""
import math
import numpy as np

import concourse.bass as bass
import concourse.bacc as bacc
import concourse.tile as tile
import concourse.mybir as mybir
from concourse.masks import make_identity
from concourse.bass_utils import run_bass_kernel_spmd

P = 128
NUM_CORES = 8
Q_NODES = 100000
NQ_CORE = Q_NODES // NUM_CORES          # 12500
OUT_DIM = 128
HIDDEN = 64
F32 = mybir.dt.float32
Alu = mybir.AluOpType
Act = mybir.ActivationFunctionType

_BUILD_CACHE = {}


def build_module(num_cores, nq_real, K, q_total, g_tiles=14):
    """Build the Bass module. nq_real = real queries per core; K = padded degree."""
    nt = math.ceil(nq_real / P)               # query tiles per core
    nq_pad = nt * P
    ngrp = math.ceil(nt / g_tiles)
    qtot_real = float(q_total)

    nc = bacc.Bacc("TRN2", target_bir_lowering=False, debug=False,
                   enable_asserts=True, num_devices=num_cores)

    NBR = nc.dram_tensor("NBR", [3, nt, P, K], F32, kind="ExternalInput")
    QP = nc.dram_tensor("QP", [3, P, nt], F32, kind="ExternalInput")
    CNT = nc.dram_tensor("CNT", [P, nt], F32, kind="ExternalInput")
    W1T = nc.dram_tensor("W1T", [9, HIDDEN], F32, kind="ExternalInput")
    B1 = nc.dram_tensor("B1", [1, HIDDEN], F32, kind="ExternalInput")
    W2T = nc.dram_tensor("W2T", [HIDDEN, OUT_DIM], F32, kind="ExternalInput")
    B2 = nc.dram_tensor("B2", [1, OUT_DIM], F32, kind="ExternalInput")
    OUT = nc.dram_tensor("OUT", [nq_pad, OUT_DIM], F32, kind="ExternalOutput")

    with tile.TileContext(nc) as tc:
        with (
            tc.tile_pool(name="cst", bufs=1) as cst,
            tc.tile_pool(name="pln", bufs=1) as pln,
            tc.tile_pool(name="big", bufs=2) as big,
            tc.tile_pool(name="eig", bufs=1) as eig,
            tc.tile_pool(name="mlp", bufs=3) as mlp,
            tc.tile_pool(name="ps", bufs=2, space="PSUM") as ps,
            tc.tile_pool(name="ps1", bufs=1, space="PSUM") as ps1,
            tc.tile_pool(name="dram", bufs=1, space="DRAM") as dram,
        ):
            # ---------- constants / small inputs ----------
            qx = cst.tile([P, nt], F32, tag="qx")
            qy = cst.tile([P, nt], F32, tag="qy")
            qz = cst.tile([P, nt], F32, tag="qz")
            cnt = cst.tile([P, nt], F32, tag="cnt")
            nc.sync.dma_start(out=qx[:], in_=QP[0])
            nc.sync.dma_start(out=qy[:], in_=QP[1])
            nc.sync.dma_start(out=qz[:], in_=QP[2])
            nc.sync.dma_start(out=cnt[:], in_=CNT[:])
            w1t = cst.tile([9, HIDDEN], F32, tag="w1t")
            b1r = cst.tile([1, HIDDEN], F32, tag="b1r")
            w2t = cst.tile([HIDDEN, OUT_DIM], F32, tag="w2t")
            b2r = cst.tile([1, OUT_DIM], F32, tag="b2r")
            nc.sync.dma_start(out=w1t[:], in_=W1T[:])
            nc.sync.dma_start(out=b1r[:], in_=B1[:])
            nc.sync.dma_start(out=w2t[:], in_=W2T[:])
            nc.sync.dma_start(out=b2r[:], in_=B2[:])
            ident = cst.tile([P, P], F32, tag="ident")
            make_identity(nc, ident[:])
            ones_col = cst.tile([P, 1], F32, tag="ones_col")
            nc.vector.memset(ones_col[:], 1.0)
            ones_row = cst.tile([1, P], F32, tag="ones_row")
            nc.vector.memset(ones_row[:], 1.0)
            bias1 = cst.tile([P, 1], F32, tag="bias1")
            nc.vector.memset(bias1[:], math.pi / 2.0)
            bias2 = cst.tile([P, 1], F32, tag="bias2")
            nc.vector.memset(bias2[:], math.pi / 6.0)

            # ---------- stat planes ----------
            def plane(tag):
                return pln.tile([P, nt], F32, tag=tag, name=tag)

            Sx, Sy, Sz = plane("Sx"), plane("Sy"), plane("Sz")
            Sxx, Syy, Szz = plane("Sxx"), plane("Syy"), plane("Szz")
            Sxy, Sxz, Syz = plane("Sxy"), plane("Sxz"), plane("Syz")
            Sd = plane("Sd")

            # ---------- phase A: per-edge work, grouped tiles ----------
            for g in range(ngrp):
                t0 = g * g_tiles
                gt = min(g_tiles, nt - t0)
                W = gt * K

                xs = big.tile([P, g_tiles, K], F32, tag="xs")
                ys = big.tile([P, g_tiles, K], F32, tag="ys")
                zs = big.tile([P, g_tiles, K], F32, tag="zs")
                nc.sync.dma_start(out=xs[:, :gt, :], in_=NBR[0, t0:t0 + gt].rearrange("j p k -> p j k"))
                nc.sync.dma_start(out=ys[:, :gt, :], in_=NBR[1, t0:t0 + gt].rearrange("j p k -> p j k"))
                nc.sync.dma_start(out=zs[:, :gt, :], in_=NBR[2, t0:t0 + gt].rearrange("j p k -> p j k"))

                # u = nbr - qpos computed in-place into xs/ys/zs
                nc.vector.tensor_tensor(out=xs[:, :gt, :], in0=xs[:, :gt, :],
                                        in1=qx[:, t0:t0 + gt].to_broadcast((P, gt, K)),
                                        op=Alu.subtract)
                nc.vector.tensor_tensor(out=ys[:, :gt, :], in0=ys[:, :gt, :],
                                        in1=qy[:, t0:t0 + gt].to_broadcast((P, gt, K)),
                                        op=Alu.subtract)
                nc.vector.tensor_tensor(out=zs[:, :gt, :], in0=zs[:, :gt, :],
                                        in1=qz[:, t0:t0 + gt].to_broadcast((P, gt, K)),
                                        op=Alu.subtract)
                ux, uy, uz = xs, ys, zs

                # squares on ACT; cross-products split DVE/GPSIMD
                xx = big.tile([P, g_tiles, K], F32, tag="xx")
                yy = big.tile([P, g_tiles, K], F32, tag="yy")
                zz = big.tile([P, g_tiles, K], F32, tag="zz")
                nc.scalar.activation(out=xx[:, :gt, :], in_=ux[:, :gt, :], func=Act.Square)
                nc.scalar.activation(out=yy[:, :gt, :], in_=uy[:, :gt, :], func=Act.Square)
                nc.scalar.activation(out=zz[:, :gt, :], in_=uz[:, :gt, :], func=Act.Square)
                xy = big.tile([P, g_tiles, K], F32, tag="xy")
                xz = big.tile([P, g_tiles, K], F32, tag="xz")
                yz = big.tile([P, g_tiles, K], F32, tag="yz")
                nc.vector.tensor_tensor(out=xy[:, :gt, :], in0=ux[:, :gt, :], in1=uy[:, :gt, :], op=Alu.mult)
                nc.gpsimd.tensor_tensor(out=xz[:, :gt, :], in0=ux[:, :gt, :], in1=uz[:, :gt, :], op=Alu.mult)
                nc.gpsimd.tensor_tensor(out=yz[:, :gt, :], in0=uy[:, :gt, :], in1=uz[:, :gt, :], op=Alu.mult)

                for srcT, dst in ((ux, Sx), (uy, Sy), (uz, Sz), (xx, Sxx), (yy, Syy),
                                  (zz, Szz), (xy, Sxy), (xz, Sxz), (yz, Syz)):
                    nc.vector.tensor_reduce(out=dst[:, t0:t0 + gt], in_=srcT[:, :gt, :],
                                            axis=mybir.AxisListType.X, op=Alu.add)

                # d2 accumulated in-place into xx (after xx/yy/zz are reduced)
                nc.gpsimd.tensor_tensor(out=xx[:, :gt, :], in0=xx[:, :gt, :], in1=yy[:, :gt, :], op=Alu.add)
                nc.gpsimd.tensor_tensor(out=xx[:, :gt, :], in0=xx[:, :gt, :], in1=zz[:, :gt, :], op=Alu.add)
                nc.scalar.activation(out=xx[:, :gt, :], in_=xx[:, :gt, :], func=Act.Sqrt)
                nc.vector.tensor_reduce(out=Sd[:, t0:t0 + gt], in_=xx[:, :gt, :],
                                        axis=mybir.AxisListType.X, op=Alu.add)

            # ---------- phase B: eigensolve + feats (planes [P, nt]) ----------
            F = eig.tile([P, nt, 32], F32, tag="F")   # feats interleaved, cols 9..31 zero
            nc.vector.memset(F[:], 0.0)
            nc.vector.memset(F[:, :, 9], 1.0)          # bias row for folded b1'

            def ev(tag):
                return eig.tile([P, nt], F32, tag=tag, name=tag)

            Ncl = ev("Ncl")
            invN = ev("invN")
            nc.vector.tensor_scalar(out=Ncl[:], in0=cnt[:], scalar1=1.0, scalar2=None, op0=Alu.max)
            nc.vector.reciprocal(out=invN[:], in_=Ncl[:])

            # f0 = counts
            nc.scalar.copy(out=F[:, :, 0], in_=cnt[:])
            # Delta = S{xyz} * invN  -> f3..f5 (also centroid-offset c)
            nc.vector.tensor_tensor(out=F[:, :, 3], in0=Sx[:], in1=invN[:], op=Alu.mult)
            nc.vector.tensor_tensor(out=F[:, :, 4], in0=Sy[:], in1=invN[:], op=Alu.mult)
            nc.vector.tensor_tensor(out=F[:, :, 5], in0=Sz[:], in1=invN[:], op=Alu.mult)
            # D_avg -> f1 ; D_var -> f2
            nc.vector.tensor_tensor(out=F[:, :, 1], in0=Sd[:], in1=invN[:], op=Alu.mult)
            sd2 = ev("sd2")
            nc.vector.tensor_tensor(out=sd2[:], in0=Sxx[:], in1=Syy[:], op=Alu.add)
            nc.vector.tensor_tensor(out=sd2[:], in0=sd2[:], in1=Szz[:], op=Alu.add)
            ex2 = ev("ex2")
            nc.vector.tensor_tensor(out=ex2[:], in0=sd2[:], in1=invN[:], op=Alu.mult)
            da2 = ev("da2")
            nc.scalar.activation(out=da2[:], in_=F[:, :, 1], func=Act.Square)
            dv = ev("dv")
            nc.vector.tensor_tensor(out=dv[:], in0=ex2[:], in1=da2[:], op=Alu.subtract)
            nc.vector.tensor_scalar(out=F[:, :, 2], in0=dv[:], scalar1=0.0, scalar2=None, op0=Alu.max)

            # cov = Suu*invN - c c^T
            cx, cy, cz = F[:, :, 3], F[:, :, 4], F[:, :, 5]
            covp = {}
            for nm, Spl, ca, cb in (("axx", Sxx, cx, cx), ("ayy", Syy, cy, cy),
                                    ("azz", Szz, cz, cz), ("axy", Sxy, cx, cy),
                                    ("axz", Sxz, cx, cz), ("ayz", Syz, cy, cz)):
                m = ev("m_" + nm)
                nc.vector.tensor_tensor(out=m[:], in0=Spl[:], in1=invN[:], op=Alu.mult)
                cc = ev("cc_" + nm)
                nc.gpsimd.tensor_tensor(out=cc[:], in0=ca, in1=cb, op=Alu.mult)
                a = ev(nm)
                nc.vector.tensor_tensor(out=a[:], in0=m[:], in1=cc[:], op=Alu.subtract)
                covp[nm] = a
            axx, ayy, azz = covp["axx"], covp["ayy"], covp["azz"]
            axy, axz, ayz = covp["axy"], covp["axz"], covp["ayz"]

            # trig closed-form eigenvalues
            q3 = ev("q3")
            nc.vector.tensor_tensor(out=q3[:], in0=axx[:], in1=ayy[:], op=Alu.add)
            nc.vector.tensor_tensor(out=q3[:], in0=q3[:], in1=azz[:], op=Alu.add)
            qq = ev("qq")
            nc.vector.tensor_scalar(out=qq[:], in0=q3[:], scalar1=1.0 / 3.0, scalar2=None, op0=Alu.mult)
            sq_xy = ev("sq_xy"); sq_xz = ev("sq_xz"); sq_yz = ev("sq_yz")
            nc.scalar.activation(out=sq_xy[:], in_=axy[:], func=Act.Square)
            nc.scalar.activation(out=sq_xz[:], in_=axz[:], func=Act.Square)
            nc.scalar.activation(out=sq_yz[:], in_=ayz[:], func=Act.Square)
            p1 = ev("p1")
            nc.vector.tensor_tensor(out=p1[:], in0=sq_xy[:], in1=sq_xz[:], op=Alu.add)
            nc.vector.tensor_tensor(out=p1[:], in0=p1[:], in1=sq_yz[:], op=Alu.add)
            aqx = ev("aqx"); aqy = ev("aqy"); aqz = ev("aqz")
            nc.vector.tensor_tensor(out=aqx[:], in0=axx[:], in1=qq[:], op=Alu.subtract)
            nc.vector.tensor_tensor(out=aqy[:], in0=ayy[:], in1=qq[:], op=Alu.subtract)
            nc.vector.tensor_tensor(out=aqz[:], in0=azz[:], in1=qq[:], op=Alu.subtract)
            s_aqx = ev("s_aqx"); s_aqy = ev("s_aqy"); s_aqz = ev("s_aqz")
            nc.scalar.activation(out=s_aqx[:], in_=aqx[:], func=Act.Square)
            nc.scalar.activation(out=s_aqy[:], in_=aqy[:], func=Act.Square)
            nc.scalar.activation(out=s_aqz[:], in_=aqz[:], func=Act.Square)
            p2 = ev("p2")
            nc.vector.tensor_tensor(out=p2[:], in0=s_aqx[:], in1=s_aqy[:], op=Alu.add)
            nc.vector.tensor_tensor(out=p2[:], in0=p2[:], in1=s_aqz[:], op=Alu.add)
            nc.vector.scalar_tensor_tensor(out=p2[:], in0=p1[:], scalar=2.0, in1=p2[:],
                                           op0=Alu.mult, op1=Alu.add)
            pp = ev("pp")
            nc.scalar.activation(out=pp[:], in_=p2[:], func=Act.Sqrt, scale=1.0 / 6.0)
            psafe = ev("psafe")
            nc.vector.tensor_scalar(out=psafe[:], in0=pp[:], scalar1=1e-30, scalar2=None, op0=Alu.max)
            pinv = ev("pinv")
            nc.vector.reciprocal(out=pinv[:], in_=psafe[:])

            B = {}
            for nm, a in (("bxx", aqx), ("byy", aqy), ("bzz", aqz),
                          ("bxy", axy), ("bxz", axz), ("byz", ayz)):
                b = ev(nm)
                nc.vector.tensor_tensor(out=b[:], in0=a[:], in1=pinv[:], op=Alu.mult)
                B[nm] = b
            t1 = ev("t1"); t2 = ev("t2"); t3 = ev("t3"); t4 = ev("t4")
            nc.vector.tensor_tensor(out=t1[:], in0=B["byy"][:], in1=B["bzz"][:], op=Alu.mult)
            nc.scalar.activation(out=t2[:], in_=B["byz"][:], func=Act.Square)
            nc.vector.tensor_tensor(out=t3[:], in0=t1[:], in1=t2[:], op=Alu.subtract)
            nc.vector.tensor_tensor(out=t4[:], in0=B["bxx"][:], in1=t3[:], op=Alu.mult)
            t5 = ev("t5"); t6 = ev("t6"); t7 = ev("t7"); t8 = ev("t8")
            nc.vector.tensor_tensor(out=t5[:], in0=B["bxy"][:], in1=B["bzz"][:], op=Alu.mult)
            nc.gpsimd.tensor_tensor(out=t6[:], in0=B["byz"][:], in1=B["bxz"][:], op=Alu.mult)
            nc.vector.tensor_tensor(out=t7[:], in0=t5[:], in1=t6[:], op=Alu.subtract)
            nc.vector.tensor_tensor(out=t8[:], in0=B["bxy"][:], in1=t7[:], op=Alu.mult)
            t9 = ev("t9"); t10 = ev("t10"); t11 = ev("t11"); t12 = ev("t12")
            nc.gpsimd.tensor_tensor(out=t9[:], in0=B["bxy"][:], in1=B["byz"][:], op=Alu.mult)
            nc.vector.tensor_tensor(out=t10[:], in0=B["byy"][:], in1=B["bxz"][:], op=Alu.mult)
            nc.vector.tensor_tensor(out=t11[:], in0=t9[:], in1=t10[:], op=Alu.subtract)
            nc.vector.tensor_tensor(out=t12[:], in0=B["bxz"][:], in1=t11[:], op=Alu.mult)
            det = ev("det")
            nc.vector.tensor_tensor(out=det[:], in0=t4[:], in1=t8[:], op=Alu.subtract)
            nc.vector.tensor_tensor(out=det[:], in0=det[:], in1=t12[:], op=Alu.add)
            r = ev("r")
            RC = 1.0 - 1e-6
            nc.vector.tensor_scalar(out=r[:], in0=det[:], scalar1=0.5, scalar2=RC,
                                    op0=Alu.mult, op1=Alu.min)
            nc.vector.tensor_scalar(out=r[:], in0=r[:], scalar1=-RC, scalar2=None, op0=Alu.max)
            # acos via Abramowitz-Stegun 4.4.46 polynomial (|err| <= 2e-8):
            # acos(x) = sqrt(1-x) * P(x), x in [0,1]; acos(-x) = pi - acos(x)
            AC = [1.5707963050, -0.2145988016, 0.0889789874, -0.0501743046,
                  0.0308918810, -0.0170881256, 0.0066700901, -0.0012624911]
            ax = ev("ax")
            nc.vector.scalar_tensor_tensor(out=ax[:], in0=r[:], scalar=-1.0, in1=r[:],
                                           op0=Alu.mult, op1=Alu.max)
            poly = ev("poly")
            nc.vector.tensor_scalar(out=poly[:], in0=ax[:], scalar1=AC[7], scalar2=AC[6],
                                    op0=Alu.mult, op1=Alu.add)
            for k in range(5, -1, -1):
                nc.vector.tensor_tensor(out=poly[:], in0=poly[:], in1=ax[:], op=Alu.mult)
                nc.vector.tensor_scalar(out=poly[:], in0=poly[:], scalar1=AC[k],
                                        scalar2=None, op0=Alu.add)
            omx = ev("omx")
            nc.vector.tensor_scalar(out=omx[:], in0=ax[:], scalar1=-1.0, scalar2=1.0,
                                    op0=Alu.mult, op1=Alu.add)
            sq1x = ev("sq1x")
            nc.scalar.activation(out=sq1x[:], in_=omx[:], func=Act.Sqrt)
            acp = ev("acp")
            nc.vector.tensor_tensor(out=acp[:], in0=poly[:], in1=sq1x[:], op=Alu.mult)
            sgn = ev("sgn")
            nc.scalar.activation(out=sgn[:], in_=r[:], func=Act.Sign)
            ach = ev("ach")
            nc.vector.tensor_scalar(out=ach[:], in0=acp[:], scalar1=-math.pi / 2.0,
                                    scalar2=None, op0=Alu.add)
            acr = ev("acr")
            nc.vector.tensor_tensor(out=acr[:], in0=sgn[:], in1=ach[:], op=Alu.mult)
            nc.vector.tensor_scalar(out=acr[:], in0=acr[:], scalar1=math.pi / 2.0,
                                    scalar2=None, op0=Alu.add)
            # cos(phi) = sin(pi/2 - phi); cos(phi+2pi/3) = -sin(phi+pi/6); phi = acr/3
            cos1 = ev("cos1"); sin2 = ev("sin2")
            nc.scalar.activation(out=cos1[:], in_=acr[:], func=Act.Sin,
                                 scale=-1.0 / 3.0, bias=bias1[:])
            nc.scalar.activation(out=sin2[:], in_=acr[:], func=Act.Sin,
                                 scale=1.0 / 3.0, bias=bias2[:])
            tp1 = ev("tp1"); tp2 = ev("tp2")
            nc.vector.tensor_tensor(out=tp1[:], in0=pp[:], in1=cos1[:], op=Alu.mult)
            nc.vector.scalar_tensor_tensor(out=F[:, :, 6], in0=tp1[:], scalar=2.0, in1=qq[:],
                                           op0=Alu.mult, op1=Alu.add)
            nc.vector.tensor_tensor(out=tp2[:], in0=pp[:], in1=sin2[:], op=Alu.mult)
            nc.vector.scalar_tensor_tensor(out=F[:, :, 8], in0=tp2[:], scalar=-2.0, in1=qq[:],
                                           op0=Alu.mult, op1=Alu.add)
            e2a = ev("e2a")
            nc.vector.scalar_tensor_tensor(out=e2a[:], in0=qq[:], scalar=3.0, in1=F[:, :, 6],
                                           op0=Alu.mult, op1=Alu.subtract)
            nc.vector.tensor_tensor(out=F[:, :, 7], in0=e2a[:], in1=F[:, :, 8], op=Alu.subtract)

            # ---------- phase C: standardization partial sums + AllReduce ----------
            S1 = eig.tile([P, 9], F32, tag="S1")
            S2 = eig.tile([P, 9], F32, tag="S2")
            sqscr = eig.tile([P, nt], F32, tag="sqscr")
            for s in range(9):
                nc.vector.tensor_reduce(out=S1[:, s:s + 1], in_=F[:, :, s],
                                        axis=mybir.AxisListType.X, op=Alu.add)
                nc.scalar.activation(out=sqscr[:], in_=F[:, :, s], func=Act.Square,
                                     accum_out=S2[:, s:s + 1])
            psS = ps1.tile([9, 2], F32, tag="psS")
            nc.tensor.matmul(out=psS[:, 0:1], lhsT=S1[:], rhs=ones_col[:], start=True, stop=True)
            nc.tensor.matmul(out=psS[:, 1:2], lhsT=S2[:], rhs=ones_col[:], start=True, stop=True)
            cpre = eig.tile([9, 2], F32, tag="cpre")
            nc.vector.tensor_copy(out=cpre[:], in_=psS[:])
            csum = eig.tile([9, 2], F32, tag="csum")
            if num_cores > 1:
                cin = dram.tile([9, 2], F32, tag="cin")
                cout = dram.tile([9, 2], F32, tag="cout")
                nc.sync.dma_start(out=cin[:], in_=cpre[:])
                nc.gpsimd.collective_compute(
                    "AllReduce", Alu.add,
                    replica_groups=[list(range(num_cores))],
                    ins=[cin.opt()], outs=[cout.opt()])
                nc.sync.dma_start(out=csum[:], in_=cout[:])
            else:
                nc.vector.tensor_copy(out=csum[:], in_=cpre[:])

            mu = eig.tile([9, 1], F32, tag="mu")
            nc.vector.tensor_scalar(out=mu[:], in0=csum[:, 0:1], scalar1=1.0 / qtot_real,
                                    scalar2=None, op0=Alu.mult)
            ex2s = eig.tile([9, 1], F32, tag="ex2s")
            nc.vector.tensor_scalar(out=ex2s[:], in0=csum[:, 1:2], scalar1=1.0 / qtot_real,
                                    scalar2=None, op0=Alu.mult)
            mu2 = eig.tile([9, 1], F32, tag="mu2")
            nc.vector.tensor_tensor(out=mu2[:], in0=mu[:], in1=mu[:], op=Alu.mult)
            varr = eig.tile([9, 1], F32, tag="varr")
            nc.vector.tensor_tensor(out=varr[:], in0=ex2s[:], in1=mu2[:], op=Alu.subtract)
            nc.vector.tensor_scalar(out=varr[:], in0=varr[:],
                                    scalar1=qtot_real / (qtot_real - 1.0), scalar2=0.0,
                                    op0=Alu.mult, op1=Alu.max)
            stdv = eig.tile([9, 1], F32, tag="stdv")
            nc.scalar.activation(out=stdv[:], in_=varr[:], func=Act.Sqrt)
            cmpm = eig.tile([9, 1], F32, tag="cmpm")
            nc.vector.tensor_scalar(out=cmpm[:], in0=stdv[:], scalar1=1e-6, scalar2=None,
                                    op0=Alu.is_ge)
            stm1 = eig.tile([9, 1], F32, tag="stm1")
            nc.vector.tensor_scalar(out=stm1[:], in0=stdv[:], scalar1=-1.0, scalar2=None,
                                    op0=Alu.add)
            stdc = eig.tile([9, 1], F32, tag="stdc")
            nc.vector.tensor_tensor(out=stdc[:], in0=cmpm[:], in1=stm1[:], op=Alu.mult)
            nc.vector.tensor_scalar(out=stdc[:], in0=stdc[:], scalar1=1.0, scalar2=None,
                                    op0=Alu.add)
            sinv9 = eig.tile([9, 1], F32, tag="sinv9")
            nc.vector.reciprocal(out=sinv9[:], in_=stdc[:])
            musv = eig.tile([9, 1], F32, tag="musv")
            nc.vector.tensor_tensor(out=musv[:], in0=mu[:], in1=sinv9[:], op=Alu.mult)

            # folded weights at partition bases 0/32/64 (+ bias row 9)
            W1s = mlp.tile([10, HIDDEN], F32, tag="W1s", bufs=1)
            nc.vector.tensor_scalar(out=W1s[0:9, :], in0=w1t[:], scalar1=sinv9[:],
                                    scalar2=None, op0=Alu.mult)
            psB = ps1.tile([1, HIDDEN], F32, tag="psB")
            nc.tensor.matmul(out=psB[:], lhsT=musv[:], rhs=w1t[:], start=True, stop=True)
            b1p = eig.tile([1, HIDDEN], F32, tag="b1p")
            nc.vector.scalar_tensor_tensor(out=b1p[:], in0=psB[:], scalar=-1.0,
                                           in1=b1r[:], op0=Alu.mult, op1=Alu.add)
            nc.sync.dma_start(out=W1s[9:10, :], in_=b1p[:])

            # ---------- phase D: feats transposes (3 tiles / transpose) ----------
            ngt = math.ceil(nt / 3)
            TSBs = []
            for j in range(3):
                TSBs.append(mlp.tile([32, ngt * P], F32, tag=f"TSB{j}",
                                     name=f"TSB{j}", bufs=1))
            TP_GRP = 4
            for j in range(3):
                gs = [g for g in range(ngt) if 3 * g + j < nt]
                tpp = None
                for idx, g in enumerate(gs):
                    sub = idx % TP_GRP
                    if sub == 0:
                        tpp = ps.tile([32, TP_GRP * P], F32, tag="tpp", name="tpp")
                    nc.tensor.transpose(out=tpp[:, sub * P:sub * P + P],
                                        in_=F[:, 3 * g + j, :], identity=ident[:])
                    if sub == TP_GRP - 1 or idx == len(gs) - 1:
                        nf = (sub + 1) * P
                        g0 = g - sub
                        nc.scalar.copy(out=TSBs[j][:, g0 * P:g0 * P + nf],
                                       in_=tpp[:, :nf])

            # ---------- phase E: MLP ----------
            MM_GRP = 4      # tiles per PSUM tile
            nmm = math.ceil(nt / MM_GRP)
            for mg in range(nmm):
                tA = mg * MM_GRP
                ntile = min(MM_GRP, nt - tA)
                h1p = ps.tile([HIDDEN, MM_GRP * P], F32, tag="h1p")
                for i in range(ntile):
                    t = tA + i
                    gg, j = divmod(t, 3)
                    nc.tensor.matmul(out=h1p[:, i * P:(i + 1) * P],
                                     lhsT=W1s[:],
                                     rhs=TSBs[j][0:10, gg * P:(gg + 1) * P],
                                     start=True, stop=True)
                h1 = mlp.tile([HIDDEN, MM_GRP * P], F32, tag="h1")
                if mg % 2 == 0:
                    nc.vector.tensor_scalar(out=h1[:, :ntile * P], in0=h1p[:, :ntile * P],
                                            scalar1=0.0, scalar2=None, op0=Alu.max)
                else:
                    nc.scalar.activation(out=h1[:, :ntile * P], in_=h1p[:, :ntile * P],
                                         func=Act.Relu)
                o2p = ps.tile([P, MM_GRP * OUT_DIM], F32, tag="o2p")
                for i in range(ntile):
                    nc.tensor.matmul(out=o2p[:, i * OUT_DIM:(i + 1) * OUT_DIM],
                                     lhsT=h1[:, i * P:(i + 1) * P], rhs=w2t[:],
                                     start=True, stop=False)
                    nc.tensor.matmul(out=o2p[:, i * OUT_DIM:(i + 1) * OUT_DIM],
                                     lhsT=b2r[:], rhs=ones_row[:],
                                     start=False, stop=True)
                osb = mlp.tile([P, MM_GRP * OUT_DIM], F32, tag="osb")
                if mg % 2 == 0:
                    nc.scalar.copy(out=osb[:, :ntile * OUT_DIM], in_=o2p[:, :ntile * OUT_DIM])
                else:
                    nc.vector.tensor_copy(out=osb[:, :ntile * OUT_DIM], in_=o2p[:, :ntile * OUT_DIM])
                nc.sync.dma_start(
                    out=OUT[tA * P:(tA + ntile) * P, :].rearrange("(i q) h -> q i h", i=ntile),
                    in_=osb[:, :ntile * OUT_DIM])

    nc.compile()
    return nc


def _prep_inputs(source_pos, query_pos, edge_index, W1, b1, W2, b2,
                 num_cores=NUM_CORES, nq_core=NQ_CORE, K=64):
    """Host-side graph partitioning + CSR padding (pure indexing)."""
    Q = query_pos.shape[0]
    E = edge_index.shape[1]
    qi = np.asarray(edge_index[0], dtype=np.int64)
    si = np.asarray(edge_index[1], dtype=np.int64)
    deg = np.bincount(qi, minlength=Q).astype(np.int64)
    K_eff = max(K, int(deg.max()))
    order = np.argsort(qi, kind="stable")
    qs = qi[order]
    ss = si[order]
    offs = np.zeros(Q + 1, dtype=np.int64)
    np.cumsum(deg, out=offs[1:])
    slot = np.arange(E, dtype=np.int64) - offs[qs]

    nbr = np.broadcast_to(query_pos[:, None, :], (Q, K_eff, 3)).copy()
    nbr[qs, slot] = source_pos[ss]

    nt = math.ceil(nq_core / P)
    nq_pad = nt * P
    in_maps = []
    W1T = np.ascontiguousarray(W1.T.astype(np.float32))        # [9, 64]
    B1 = np.ascontiguousarray(b1[None, :].astype(np.float32))
    W2T = np.ascontiguousarray(W2.T.astype(np.float32))        # [64, 128]
    B2 = np.ascontiguousarray(b2[None, :].astype(np.float32))
    for c in range(num_cores):
        lo = c * nq_core
        nb = np.zeros((nq_pad, K_eff, 3), dtype=np.float32)
        nb[:nq_core] = nbr[lo:lo + nq_core]
        NBR = np.ascontiguousarray(nb.reshape(nt, P, K_eff, 3).transpose(3, 0, 1, 2))
        qp = np.zeros((nq_pad, 3), dtype=np.float32)
        qp[:nq_core] = query_pos[lo:lo + nq_core]
        QP = np.ascontiguousarray(qp.reshape(nt, P, 3).transpose(2, 1, 0))
        cn = np.zeros(nq_pad, dtype=np.float32)
        cn[:nq_core] = deg[lo:lo + nq_core].astype(np.float32)
        CNT = np.ascontiguousarray(cn.reshape(nt, P).T)
        in_maps.append({"NBR": NBR, "QP": QP, "CNT": CNT,
                        "W1T": W1T, "B1": B1, "W2T": W2T, "B2": B2})
    return in_maps, K_eff


def kernel(source_pos, query_pos, edge_index, W1, b1, W2, b2):
    source_pos = np.asarray(source_pos, dtype=np.float32)
    query_pos = np.asarray(query_pos, dtype=np.float32)
    W1 = np.asarray(W1, dtype=np.float32)
    b1 = np.asarray(b1, dtype=np.float32)
    W2 = np.asarray(W2, dtype=np.float32)
    b2 = np.asarray(b2, dtype=np.float32)
    edge_index = np.asarray(edge_index)

    in_maps, K_eff = _prep_inputs(source_pos, query_pos, edge_index, W1, b1, W2, b2)
    key = (NUM_CORES, NQ_CORE, K_eff, Q_NODES)
    if key not in _BUILD_CACHE:
        _BUILD_CACHE[key] = build_module(NUM_CORES, NQ_CORE, K_eff, Q_NODES)
    nc = _BUILD_CACHE[key]
    res = run_bass_kernel_spmd(nc, in_maps, core_ids=list(range(NUM_CORES)))
    out = np.concatenate([res.results[c]["OUT"][:NQ_CORE] for c in range(NUM_CORES)], axis=0)
    return out.astype(np.float32)



# revision 6
# speedup vs baseline: 1.7793x; 1.7793x over previous
"""Trainium2 Bass kernel for nn_GeometricEmbedding (GNN message passing).

Strategy (8 NeuronCores, degree-balanced graph partition of queries):
  * Host (indexing only): global degree-sort of queries, dealt round-robin to
    the 8 cores so every core sees an identical degree profile; per-core CSR
    pad each query's neighbor list to the *per-tile-group* max degree
    (rounded to a multiple of 8) instead of a global K -> ~1.8x fewer padded
    slots than uniform K=64.  Neighbor coords shipped as packed fp16 planes
    [128, slots] (contiguous per partition => 1 descriptor/partition DMAs).
  * Device per core, phase A (per-edge): u = nbr - qpos (DVE), squares on
    ACT, cross terms split DVE/GPSIMD, d = |u| on ACT; per-query segment
    sums via an fp16 halving fold tree (DVE 2x mode) finished by a single
    fp32 tensor_reduce.
  * Phase B: 10 sum planes PE-transposed to [tiles, 128] layout; closed-form
    3x3 symmetric eigensolve (trig method) in fp32; 9 fp16 feature planes.
  * Phase C: feature mean/std partial sums + AllReduce over cores; the
    standardization is folded into W1/b1 (b1 applied as the per-partition
    bias of the fused bias+relu tensor_scalar).
  * Phase E: fp16 MLP on the tensor engine, 512-query blocks, outputs
    written transposed [OUT_DIM, nq] in fp16; host casts/permutes back.
"""
import math
import numpy as np

import concourse.bass as bass
import concourse.bacc as bacc
import concourse.tile as tile
import concourse.mybir as mybir
from concourse.masks import make_identity
from concourse.bass_utils import run_bass_kernel_spmd

P = 128
NUM_CORES = 8
Q_NODES = 100000
NQ_CORE = Q_NODES // NUM_CORES          # 12500
NT = math.ceil(NQ_CORE / P)             # 98
NQ_PAD = NT * P                         # 12544
OUT_DIM = 128
HIDDEN = 64
F32 = mybir.dt.float32
F16 = mybir.dt.float16
Alu = mybir.AluOpType
Act = mybir.ActivationFunctionType
AxX = mybir.AxisListType.X

_BUILD_CACHE = {}


def build_module(num_cores, groups, q_total=Q_NODES):
    """groups: tuple of (gt, K) tile groups covering NT tiles."""
    nt = NT
    qtot_real = float(q_total)
    slots = sum(gt * K for gt, K in groups)

    nc = bacc.Bacc("TRN2", target_bir_lowering=False, debug=False,
                   enable_asserts=True, num_devices=num_cores)

    NBRX = nc.dram_tensor("NBRX", [P, slots], F16, kind="ExternalInput")
    NBRY = nc.dram_tensor("NBRY", [P, slots], F16, kind="ExternalInput")
    NBRZ = nc.dram_tensor("NBRZ", [P, slots], F16, kind="ExternalInput")
    QP = nc.dram_tensor("QP", [3, P, nt], F32, kind="ExternalInput")
    CNTT = nc.dram_tensor("CNTT", [nt, P], F32, kind="ExternalInput")
    W1T = nc.dram_tensor("W1T", [9, HIDDEN], F32, kind="ExternalInput")
    B1C = nc.dram_tensor("B1C", [HIDDEN, 1], F32, kind="ExternalInput")
    W2TH = nc.dram_tensor("W2TH", [2 * HIDDEN, OUT_DIM], F16, kind="ExternalInput")
    B2R = nc.dram_tensor("B2R", [1, OUT_DIM], F16, kind="ExternalInput")
    OUT = nc.dram_tensor("OUT", [OUT_DIM, NQ_PAD], F16, kind="ExternalOutput")

    NBR = (NBRX, NBRY, NBRZ)

    with tile.TileContext(nc) as tc:
        with (
            tc.tile_pool(name="cst", bufs=1) as cst,
            tc.tile_pool(name="big", bufs=2) as bigp,
            tc.tile_pool(name="sums", bufs=1) as sums_p,
            tc.tile_pool(name="tsp", bufs=1) as tsp,
            tc.tile_pool(name="mlp", bufs=2) as mlp,
            tc.tile_pool(name="pst", bufs=2, space="PSUM") as pst,
            tc.tile_pool(name="ph1", bufs=2, space="PSUM") as ph1,
            tc.tile_pool(name="po2", bufs=2, space="PSUM") as po2,
            tc.tile_pool(name="pw", bufs=1, space="PSUM") as pw,
            tc.tile_pool(name="dram", bufs=1, space="DRAM") as dram,
        ):
            # ---------- constants / small inputs ----------
            qx = cst.tile([P, nt], F32, tag="qx", name="qx")
            qy = cst.tile([P, nt], F32, tag="qy", name="qy")
            qz = cst.tile([P, nt], F32, tag="qz", name="qz")
            nc.sync.dma_start(out=qx[:], in_=QP[0])
            nc.sync.dma_start(out=qy[:], in_=QP[1])
            nc.sync.dma_start(out=qz[:], in_=QP[2])
            w1t = cst.tile([9, HIDDEN], F32, tag="w1t", name="w1t")
            b1c = cst.tile([HIDDEN, 1], F32, tag="b1c", name="b1c")
            w2t = cst.tile([2 * HIDDEN, OUT_DIM], F16, tag="w2t", name="w2t")
            b2r = cst.tile([1, OUT_DIM], F16, tag="b2r", name="b2r")
            nc.sync.dma_start(out=w1t[:], in_=W1T[:])
            nc.sync.dma_start(out=b1c[:], in_=B1C[:])
            nc.sync.dma_start(out=w2t[:], in_=W2TH[:])
            nc.sync.dma_start(out=b2r[:], in_=B2R[:])
            ident = cst.tile([P, P], F32, tag="ident", name="ident")
            make_identity(nc, ident[:])
            ones_col = cst.tile([nt, 1], F32, tag="ones_col", name="ones_col")
            nc.vector.memset(ones_col[:], 1.0)
            ones16 = cst.tile([1, 512], F16, tag="ones16", name="ones16")
            nc.vector.memset(ones16[:], 1.0)
            bias1 = cst.tile([nt, 1], F32, tag="bias1", name="bias1")
            nc.vector.memset(bias1[:], math.pi / 2.0)
            bias2 = cst.tile([nt, 1], F32, tag="bias2", name="bias2")
            nc.vector.memset(bias2[:], math.pi / 6.0)

            SUMS = sums_p.tile([P, 10, nt], F32, tag="SUMS", name="SUMS")

            # ---------- phase A: per-edge work ----------
            # BU moment slots: 0 ux 1 uy 2 uz 3 xx 4 yy 5 zz 6 xy 7 xz 8 yz
            #                  9 d | 10 d2 (not folded)
            t0 = 0
            soff = 0
            for gi, (gt, K) in enumerate(groups):
                BU = bigp.tile([P, 11, gt, K], F16, tag=f"bg{K}_{gt}",
                               name=f"bg{K}_{gt}")
                for m in range(3):
                    nc.sync.dma_start(
                        out=BU[:, m],
                        in_=NBR[m][:, soff:soff + gt * K].rearrange(
                            "p (t k) -> p t k", t=gt))
                for m, qc in ((0, qx), (1, qy), (2, qz)):
                    nc.vector.tensor_tensor(
                        out=BU[:, m], in0=BU[:, m],
                        in1=qc[:, t0:t0 + gt].to_broadcast((P, gt, K)),
                        op=Alu.subtract)
                for m in range(3):
                    nc.scalar.activation(out=BU[:, 3 + m], in_=BU[:, m],
                                         func=Act.Square)
                nc.vector.tensor_tensor(out=BU[:, 6], in0=BU[:, 0],
                                        in1=BU[:, 1], op=Alu.mult)
                nc.gpsimd.tensor_tensor(out=BU[:, 7], in0=BU[:, 0],
                                        in1=BU[:, 2], op=Alu.mult)
                nc.gpsimd.tensor_tensor(out=BU[:, 8], in0=BU[:, 1],
                                        in1=BU[:, 2], op=Alu.mult)
                nc.vector.tensor_tensor(out=BU[:, 10], in0=BU[:, 3],
                                        in1=BU[:, 4], op=Alu.add)
                nc.vector.tensor_tensor(out=BU[:, 10], in0=BU[:, 10],
                                        in1=BU[:, 5], op=Alu.add)
                nc.scalar.activation(out=BU[:, 9], in_=BU[:, 10], func=Act.Sqrt)
                w = K
                while w > 6 and w % 2 == 0:
                    h = w // 2
                    nc.vector.tensor_tensor(out=BU[:, 0:10, :, 0:h],
                                            in0=BU[:, 0:10, :, 0:h],
                                            in1=BU[:, 0:10, :, h:w], op=Alu.add)
                    w = h
                nc.vector.tensor_reduce(out=SUMS[:, :, t0:t0 + gt],
                                        in_=BU[:, 0:10, :, 0:w],
                                        axis=AxX, op=Alu.add)
                t0 += gt
                soff += gt * K

            # ---------- transposes: SUMS planes -> [nt, P] ----------
            TS = []
            for m in range(10):
                pt = pst.tile([nt, P], F32, tag="pt", name="pt")
                nc.tensor.transpose(out=pt[:], in_=SUMS[:, m], identity=ident[:])
                tsm = tsp.tile([nt, P], F32, tag=f"ts{m}", name=f"ts{m}")
                if m % 2 == 0:
                    nc.scalar.copy(out=tsm[:], in_=pt[:])
                else:
                    nc.vector.tensor_copy(out=tsm[:], in_=pt[:])
                TS.append(tsm)
            Sx, Sy, Sz, Sxx, Syy, Szz, Sxy, Sxz, Syz, Sd = TS
            cnt = tsp.tile([nt, P], F32, tag="cnt", name="cnt")
            nc.sync.dma_start(out=cnt[:], in_=CNTT[:])

            # ---------- phase B: per-query features (layout [nt, P]) ----------
            def ev(tag):
                return tsp.tile([nt, P], F32, tag=tag, name=tag)

            def f16t(tag):
                return tsp.tile([nt, P], F16, tag=tag, name=tag)

            Ncl = ev("Ncl")
            invN = ev("invN")
            nc.vector.tensor_scalar(out=Ncl[:], in0=cnt[:], scalar1=1.0,
                                    scalar2=None, op0=Alu.max)
            nc.vector.reciprocal(out=invN[:], in_=Ncl[:])

            F = [f16t(f"f{s}") for s in range(9)]
            # f0 = counts
            nc.scalar.copy(out=F[0][:], in_=cnt[:])
            # D_avg
            davg = ev("davg")
            nc.vector.tensor_tensor(out=davg[:], in0=Sd[:], in1=invN[:], op=Alu.mult)
            nc.scalar.copy(out=F[1][:], in_=davg[:])
            # D_var
            sd2 = ev("sd2")
            nc.gpsimd.tensor_tensor(out=sd2[:], in0=Sxx[:], in1=Syy[:], op=Alu.add)
            nc.gpsimd.tensor_tensor(out=sd2[:], in0=sd2[:], in1=Szz[:], op=Alu.add)
            ex2 = ev("ex2")
            nc.vector.tensor_tensor(out=ex2[:], in0=sd2[:], in1=invN[:], op=Alu.mult)
            da2 = ev("da2")
            nc.scalar.activation(out=da2[:], in_=davg[:], func=Act.Square)
            dv = ev("dv")
            nc.vector.tensor_tensor(out=dv[:], in0=ex2[:], in1=da2[:], op=Alu.subtract)
            nc.vector.tensor_scalar(out=F[2][:], in0=dv[:], scalar1=0.0,
                                    scalar2=None, op0=Alu.max)
            # Delta = mean(u) = centroid - qpos
            cx, cy, cz = ev("cx"), ev("cy"), ev("cz")
            nc.vector.tensor_tensor(out=cx[:], in0=Sx[:], in1=invN[:], op=Alu.mult)
            nc.vector.tensor_tensor(out=cy[:], in0=Sy[:], in1=invN[:], op=Alu.mult)
            nc.vector.tensor_tensor(out=cz[:], in0=Sz[:], in1=invN[:], op=Alu.mult)
            nc.scalar.copy(out=F[3][:], in_=cx[:])
            nc.scalar.copy(out=F[4][:], in_=cy[:])
            nc.scalar.copy(out=F[5][:], in_=cz[:])

            # cov = S(uu)/N - c c^T
            covp = {}
            for nm, Spl, ca, cb in (("axx", Sxx, cx, cx), ("ayy", Syy, cy, cy),
                                    ("azz", Szz, cz, cz), ("axy", Sxy, cx, cy),
                                    ("axz", Sxz, cx, cz), ("ayz", Syz, cy, cz)):
                m = ev("m_" + nm)
                nc.vector.tensor_tensor(out=m[:], in0=Spl[:], in1=invN[:], op=Alu.mult)
                cc = ev("cc_" + nm)
                nc.gpsimd.tensor_tensor(out=cc[:], in0=ca[:], in1=cb[:], op=Alu.mult)
                a = ev(nm)
                nc.vector.tensor_tensor(out=a[:], in0=m[:], in1=cc[:], op=Alu.subtract)
                covp[nm] = a
            axx, ayy, azz = covp["axx"], covp["ayy"], covp["azz"]
            axy, axz, ayz = covp["axy"], covp["axz"], covp["ayz"]

            # trig closed-form eigenvalues of symmetric 3x3
            q3 = ev("q3")
            nc.vector.tensor_tensor(out=q3[:], in0=axx[:], in1=ayy[:], op=Alu.add)
            nc.vector.tensor_tensor(out=q3[:], in0=q3[:], in1=azz[:], op=Alu.add)
            qq = ev("qq")
            nc.vector.tensor_scalar(out=qq[:], in0=q3[:], scalar1=1.0 / 3.0,
                                    scalar2=None, op0=Alu.mult)
            sq_xy = ev("sq_xy"); sq_xz = ev("sq_xz"); sq_yz = ev("sq_yz")
            nc.scalar.activation(out=sq_xy[:], in_=axy[:], func=Act.Square)
            nc.scalar.activation(out=sq_xz[:], in_=axz[:], func=Act.Square)
            nc.scalar.activation(out=sq_yz[:], in_=ayz[:], func=Act.Square)
            p1 = ev("p1")
            nc.vector.tensor_tensor(out=p1[:], in0=sq_xy[:], in1=sq_xz[:], op=Alu.add)
            nc.vector.tensor_tensor(out=p1[:], in0=p1[:], in1=sq_yz[:], op=Alu.add)
            aqx = ev("aqx"); aqy = ev("aqy"); aqz = ev("aqz")
            nc.vector.tensor_tensor(out=aqx[:], in0=axx[:], in1=qq[:], op=Alu.subtract)
            nc.vector.tensor_tensor(out=aqy[:], in0=ayy[:], in1=qq[:], op=Alu.subtract)
            nc.vector.tensor_tensor(out=aqz[:], in0=azz[:], in1=qq[:], op=Alu.subtract)
            s_aqx = ev("s_aqx"); s_aqy = ev("s_aqy"); s_aqz = ev("s_aqz")
            nc.scalar.activation(out=s_aqx[:], in_=aqx[:], func=Act.Square)
            nc.scalar.activation(out=s_aqy[:], in_=aqy[:], func=Act.Square)
            nc.scalar.activation(out=s_aqz[:], in_=aqz[:], func=Act.Square)
            p2 = ev("p2")
            nc.vector.tensor_tensor(out=p2[:], in0=s_aqx[:], in1=s_aqy[:], op=Alu.add)
            nc.vector.tensor_tensor(out=p2[:], in0=p2[:], in1=s_aqz[:], op=Alu.add)
            nc.vector.scalar_tensor_tensor(out=p2[:], in0=p1[:], scalar=2.0,
                                           in1=p2[:], op0=Alu.mult, op1=Alu.add)
            pp = ev("pp")
            nc.scalar.activation(out=pp[:], in_=p2[:], func=Act.Sqrt, scale=1.0 / 6.0)
            psafe = ev("psafe")
            nc.vector.tensor_scalar(out=psafe[:], in0=pp[:], scalar1=1e-30,
                                    scalar2=None, op0=Alu.max)
            pinv = ev("pinv")
            nc.vector.reciprocal(out=pinv[:], in_=psafe[:])

            B = {}
            for nm, a in (("bxx", aqx), ("byy", aqy), ("bzz", aqz),
                          ("bxy", axy), ("bxz", axz), ("byz", ayz)):
                b = ev(nm)
                nc.vector.tensor_tensor(out=b[:], in0=a[:], in1=pinv[:], op=Alu.mult)
                B[nm] = b
            t1 = ev("t1"); t2 = ev("t2"); t3 = ev("t3"); t4 = ev("t4")
            nc.vector.tensor_tensor(out=t1[:], in0=B["byy"][:], in1=B["bzz"][:], op=Alu.mult)
            nc.scalar.activation(out=t2[:], in_=B["byz"][:], func=Act.Square)
            nc.vector.tensor_tensor(out=t3[:], in0=t1[:], in1=t2[:], op=Alu.subtract)
            nc.vector.tensor_tensor(out=t4[:], in0=B["bxx"][:], in1=t3[:], op=Alu.mult)
            t5 = ev("t5"); t6 = ev("t6"); t7 = ev("t7"); t8 = ev("t8")
            nc.vector.tensor_tensor(out=t5[:], in0=B["bxy"][:], in1=B["bzz"][:], op=Alu.mult)
            nc.gpsimd.tensor_tensor(out=t6[:], in0=B["byz"][:], in1=B["bxz"][:], op=Alu.mult)
            nc.vector.tensor_tensor(out=t7[:], in0=t5[:], in1=t6[:], op=Alu.subtract)
            nc.vector.tensor_tensor(out=t8[:], in0=B["bxy"][:], in1=t7[:], op=Alu.mult)
            t9 = ev("t9"); t10 = ev("t10"); t11 = ev("t11"); t12 = ev("t12")
            nc.gpsimd.tensor_tensor(out=t9[:], in0=B["bxy"][:], in1=B["byz"][:], op=Alu.mult)
            nc.vector.tensor_tensor(out=t10[:], in0=B["byy"][:], in1=B["bxz"][:], op=Alu.mult)
            nc.vector.tensor_tensor(out=t11[:], in0=t9[:], in1=t10[:], op=Alu.subtract)
            nc.vector.tensor_tensor(out=t12[:], in0=B["bxz"][:], in1=t11[:], op=Alu.mult)
            det = ev("det")
            nc.vector.tensor_tensor(out=det[:], in0=t4[:], in1=t8[:], op=Alu.subtract)
            nc.vector.tensor_tensor(out=det[:], in0=det[:], in1=t12[:], op=Alu.add)
            r = ev("r")
            RC = 1.0 - 1e-6
            nc.vector.tensor_scalar(out=r[:], in0=det[:], scalar1=0.5, scalar2=RC,
                                    op0=Alu.mult, op1=Alu.min)
            nc.vector.tensor_scalar(out=r[:], in0=r[:], scalar1=-RC, scalar2=None,
                                    op0=Alu.max)
            # acos via Abramowitz-Stegun 4.4.46 polynomial
            AC = [1.5707963050, -0.2145988016, 0.0889789874, -0.0501743046,
                  0.0308918810, -0.0170881256, 0.0066700901, -0.0012624911]
            ax = ev("ax")
            nc.vector.scalar_tensor_tensor(out=ax[:], in0=r[:], scalar=-1.0,
                                           in1=r[:], op0=Alu.mult, op1=Alu.max)
            poly = ev("poly")
            nc.vector.tensor_scalar(out=poly[:], in0=ax[:], scalar1=AC[7],
                                    scalar2=AC[6], op0=Alu.mult, op1=Alu.add)
            for k in range(5, -1, -1):
                nc.vector.tensor_tensor(out=poly[:], in0=poly[:], in1=ax[:], op=Alu.mult)
                nc.vector.tensor_scalar(out=poly[:], in0=poly[:], scalar1=AC[k],
                                        scalar2=None, op0=Alu.add)
            omx = ev("omx")
            nc.vector.tensor_scalar(out=omx[:], in0=ax[:], scalar1=-1.0, scalar2=1.0,
                                    op0=Alu.mult, op1=Alu.add)
            sq1x = ev("sq1x")
            nc.scalar.activation(out=sq1x[:], in_=omx[:], func=Act.Sqrt)
            acp = ev("acp")
            nc.vector.tensor_tensor(out=acp[:], in0=poly[:], in1=sq1x[:], op=Alu.mult)
            sgn = ev("sgn")
            nc.scalar.activation(out=sgn[:], in_=r[:], func=Act.Sign)
            ach = ev("ach")
            nc.vector.tensor_scalar(out=ach[:], in0=acp[:], scalar1=-math.pi / 2.0,
                                    scalar2=None, op0=Alu.add)
            acr = ev("acr")
            nc.vector.tensor_tensor(out=acr[:], in0=sgn[:], in1=ach[:], op=Alu.mult)
            nc.vector.tensor_scalar(out=acr[:], in0=acr[:], scalar1=math.pi / 2.0,
                                    scalar2=None, op0=Alu.add)
            # cos(phi) = sin(pi/2 - phi); cos(phi+2pi/3) = -sin(phi+pi/6)
            cos1 = ev("cos1"); sin2 = ev("sin2")
            nc.scalar.activation(out=cos1[:], in_=acr[:], func=Act.Sin,
                                 scale=-1.0 / 3.0, bias=bias1[:])
            nc.scalar.activation(out=sin2[:], in_=acr[:], func=Act.Sin,
                                 scale=1.0 / 3.0, bias=bias2[:])
            e1 = ev("e1"); e3 = ev("e3")
            tp1 = ev("tp1"); tp2 = ev("tp2")
            nc.vector.tensor_tensor(out=tp1[:], in0=pp[:], in1=cos1[:], op=Alu.mult)
            nc.vector.scalar_tensor_tensor(out=e1[:], in0=tp1[:], scalar=2.0,
                                           in1=qq[:], op0=Alu.mult, op1=Alu.add)
            nc.vector.tensor_tensor(out=tp2[:], in0=pp[:], in1=sin2[:], op=Alu.mult)
            nc.vector.scalar_tensor_tensor(out=e3[:], in0=tp2[:], scalar=-2.0,
                                           in1=qq[:], op0=Alu.mult, op1=Alu.add)
            e2 = ev("e2")
            nc.vector.scalar_tensor_tensor(out=e2[:], in0=qq[:], scalar=3.0,
                                           in1=e1[:], op0=Alu.mult, op1=Alu.subtract)
            nc.scalar.copy(out=F[6][:], in_=e1[:])
            nc.vector.tensor_tensor(out=F[7][:], in0=e2[:], in1=e3[:], op=Alu.subtract)
            nc.scalar.copy(out=F[8][:], in_=e3[:])

            # ---------- phase C: standardization stats + AllReduce ----------
            S12 = tsp.tile([nt, 18], F32, tag="S12", name="S12")
            sqscr = tsp.tile([nt, P], F16, tag="sqscr", name="sqscr")
            for s in range(9):
                nc.vector.tensor_reduce(out=S12[:, s:s + 1], in_=F[s][:],
                                        axis=AxX, op=Alu.add)
                nc.scalar.activation(out=sqscr[:], in_=F[s][:], func=Act.Square,
                                     accum_out=S12[:, 9 + s:10 + s])
            psS = pw.tile([9, 2], F32, tag="psS", name="psS")
            nc.tensor.matmul(out=psS[:, 0:1], lhsT=S12[:, 0:9], rhs=ones_col[:],
                             start=True, stop=True)
            nc.tensor.matmul(out=psS[:, 1:2], lhsT=S12[:, 9:18], rhs=ones_col[:],
                             start=True, stop=True)
            cpre = tsp.tile([9, 2], F32, tag="cpre", name="cpre")
            nc.vector.tensor_copy(out=cpre[:], in_=psS[:])
            csum = tsp.tile([9, 2], F32, tag="csum", name="csum")
            if num_cores > 1:
                cin = dram.tile([9, 2], F32, tag="cin", name="cin")
                cout = dram.tile([9, 2], F32, tag="cout", name="cout")
                nc.sync.dma_start(out=cin[:], in_=cpre[:])
                nc.gpsimd.collective_compute(
                    "AllReduce", Alu.add,
                    replica_groups=[list(range(num_cores))],
                    ins=[cin.opt()], outs=[cout.opt()])
                nc.sync.dma_start(out=csum[:], in_=cout[:])
            else:
                nc.vector.tensor_copy(out=csum[:], in_=cpre[:])

            mu = tsp.tile([9, 1], F32, tag="mu", name="mu")
            nc.vector.tensor_scalar(out=mu[:], in0=csum[:, 0:1],
                                    scalar1=1.0 / qtot_real, scalar2=None, op0=Alu.mult)
            ex2s = tsp.tile([9, 1], F32, tag="ex2s", name="ex2s")
            nc.vector.tensor_scalar(out=ex2s[:], in0=csum[:, 1:2],
                                    scalar1=1.0 / qtot_real, scalar2=None, op0=Alu.mult)
            mu2 = tsp.tile([9, 1], F32, tag="mu2", name="mu2")
            nc.vector.tensor_tensor(out=mu2[:], in0=mu[:], in1=mu[:], op=Alu.mult)
            varr = tsp.tile([9, 1], F32, tag="varr", name="varr")
            nc.vector.tensor_tensor(out=varr[:], in0=ex2s[:], in1=mu2[:], op=Alu.subtract)
            nc.vector.tensor_scalar(out=varr[:], in0=varr[:],
                                    scalar1=qtot_real / (qtot_real - 1.0),
                                    scalar2=0.0, op0=Alu.mult, op1=Alu.max)
            stdv = tsp.tile([9, 1], F32, tag="stdv", name="stdv")
            nc.scalar.activation(out=stdv[:], in_=varr[:], func=Act.Sqrt)
            cmpm = tsp.tile([9, 1], F32, tag="cmpm", name="cmpm")
            nc.vector.tensor_scalar(out=cmpm[:], in0=stdv[:], scalar1=1e-6,
                                    scalar2=None, op0=Alu.is_ge)
            stm1 = tsp.tile([9, 1], F32, tag="stm1", name="stm1")
            nc.vector.tensor_scalar(out=stm1[:], in0=stdv[:], scalar1=-1.0,
                                    scalar2=None, op0=Alu.add)
            stdc = tsp.tile([9, 1], F32, tag="stdc", name="stdc")
            nc.vector.tensor_tensor(out=stdc[:], in0=cmpm[:], in1=stm1[:], op=Alu.mult)
            nc.vector.tensor_scalar(out=stdc[:], in0=stdc[:], scalar1=1.0,
                                    scalar2=None, op0=Alu.add)
            sinv9 = tsp.tile([9, 1], F32, tag="sinv9", name="sinv9")
            nc.vector.reciprocal(out=sinv9[:], in_=stdc[:])
            musv = tsp.tile([9, 1], F32, tag="musv", name="musv")
            nc.vector.tensor_tensor(out=musv[:], in0=mu[:], in1=sinv9[:], op=Alu.mult)

            # W1s = diag(1/std) @ W1^T in fp16; b1' = b1 - W1 @ (mu/std) as column
            W1s = mlp.tile([9, HIDDEN], F16, tag="W1s", name="W1s", bufs=1)
            nc.vector.tensor_scalar(out=W1s[:], in0=w1t[:], scalar1=sinv9[:],
                                    scalar2=None, op0=Alu.mult)
            psBc = pw.tile([HIDDEN, 1], F32, tag="psBc", name="psBc")
            nc.tensor.matmul(out=psBc[:], lhsT=w1t[:], rhs=musv[:],
                             start=True, stop=True)
            b1c2 = mlp.tile([P, 1], F32, tag="b1c2", name="b1c2", bufs=1)
            nc.vector.scalar_tensor_tensor(out=b1c2[0:HIDDEN], in0=psBc[:],
                                           scalar=-1.0, in1=b1c[:],
                                           op0=Alu.mult, op1=Alu.add)
            nc.sync.dma_start(out=b1c2[HIDDEN:P], in_=b1c2[0:HIDDEN])

            # ---------- FT assembly: [9, NQ_PAD] fp16 ----------
            FT = mlp.tile([9, NQ_PAD], F16, tag="FT", name="FT", bufs=1)
            for s in range(9):
                nc.sync.dma_start(out=FT[s:s + 1, :], in_=F[s][:])

            # ---------- phase E: MLP ----------
            nblk = math.ceil(NQ_PAD / 1024)       # 13 (last block 256 cols)
            for j in range(nblk):
                q0 = j * 1024
                h1p = ph1.tile([P, 512], F32, tag="h1p", name="h1p")
                widths = []
                for half in range(2):
                    qa = q0 + half * 512
                    w = min(512, NQ_PAD - qa)
                    widths.append(w)
                    if w <= 0:
                        continue
                    nc.tensor.matmul(out=h1p[half * HIDDEN:(half + 1) * HIDDEN, :w],
                                     lhsT=W1s[:], rhs=FT[:, qa:qa + w],
                                     start=True, stop=True)
                wmax = max(widths)
                h1 = mlp.tile([P, 512], F16, tag="h1", name="h1")
                nc.vector.tensor_scalar(out=h1[:, :wmax], in0=h1p[:, :wmax],
                                        scalar1=b1c2[:], scalar2=0.0,
                                        op0=Alu.add, op1=Alu.max)
                osb = mlp.tile([P, 1024], F16, tag="osb", name="osb")
                for half in range(2):
                    qa = q0 + half * 512
                    w = widths[half]
                    if w <= 0:
                        continue
                    o2 = po2.tile([P, 512], F32, tag="o2", name="o2")
                    nc.tensor.matmul(out=o2[:, :w], lhsT=b2r[:], rhs=ones16[:, :w],
                                     start=True, stop=False)
                    nc.tensor.matmul(out=o2[:, :w],
                                     lhsT=w2t[half * HIDDEN:(half + 1) * HIDDEN, :],
                                     rhs=h1[half * HIDDEN:(half + 1) * HIDDEN, :w],
                                     start=False, stop=True)
                    if half % 2 == 0:
                        nc.scalar.copy(out=osb[:, half * 512:half * 512 + w],
                                       in_=o2[:, :w])
                    else:
                        nc.vector.tensor_copy(out=osb[:, half * 512:half * 512 + w],
                                              in_=o2[:, :w])
                wtot = sum(w for w in widths if w > 0)
                nc.sync.dma_start(out=OUT[:, q0:q0 + wtot], in_=osb[:, :wtot])

    nc.compile()
    return nc


def _roundup8(x):
    return max(8, (int(x) + 7) // 8 * 8)


def _make_groups(K_t, gmax=12, slack=8):
    """Greedy grouping of the (non-increasing) per-tile K profile."""
    groups = []
    t = 0
    while t < NT:
        K = K_t[t]
        gt = 1
        while t + gt < NT and gt < gmax and K - K_t[t + gt] <= slack:
            gt += 1
        groups.append((gt, K))
        t += gt
    return tuple(groups)


def _prep_inputs(source_pos, query_pos, edge_index, W1, b1, W2, b2):
    """Host-side graph partitioning + packing (indexing / layout only)."""
    Q = query_pos.shape[0]
    E = edge_index.shape[1]
    qi = np.asarray(edge_index[0], dtype=np.int64)
    si = np.asarray(edge_index[1], dtype=np.int64)
    deg = np.bincount(qi, minlength=Q).astype(np.int64)

    order = np.argsort(-deg, kind="stable")
    perms = [order[c::NUM_CORES] for c in range(NUM_CORES)]

    # per-tile max degree across cores -> shared K profile
    K_t = np.zeros(NT, dtype=np.int64)
    for t in range(NT):
        mx = 8
        for c in range(NUM_CORES):
            seg = perms[c][t * P:(t + 1) * P]
            if seg.size:
                mx = max(mx, int(deg[seg].max()))
        K_t[t] = _roundup8(mx)
    groups = _make_groups(K_t)

    # CSR over edges sorted by query
    eorder = np.argsort(qi, kind="stable")
    ssort = si[eorder]
    offs = np.zeros(Q + 1, dtype=np.int64)
    np.cumsum(deg, out=offs[1:])

    qpr = query_pos.astype(np.float16).astype(np.float32)

    W1T = np.ascontiguousarray(W1.T.astype(np.float32))            # [9, 64]
    B1C = np.ascontiguousarray(b1[:, None].astype(np.float32))     # [64, 1]
    W2TH = np.ascontiguousarray(
        np.concatenate([W2.T, W2.T], axis=0).astype(np.float16))   # [128, 128]
    B2R = np.ascontiguousarray(b2[None, :].astype(np.float16))     # [1, 128]

    in_maps = []
    for c in range(NUM_CORES):
        q_pad = np.full(NQ_PAD, -1, dtype=np.int64)
        q_pad[:NQ_CORE] = perms[c]
        valid_q = q_pad >= 0
        q_safe = np.where(valid_q, q_pad, 0)
        deg_l = np.where(valid_q, deg[q_safe], 0)
        qp_l = np.where(valid_q[:, None], qpr[q_safe], 0.0).astype(np.float32)

        planes = [[], [], []]
        t0 = 0
        for gt, K in groups:
            jj = slice(t0 * P, (t0 + gt) * P)
            dg = deg_l[jj]
            base = offs[q_safe[jj]] * valid_q[jj]
            karr = np.arange(K, dtype=np.int64)
            gidx = np.minimum(base[:, None] + karr[None, :], E - 1)
            mask = karr[None, :] < dg[:, None]
            coords = source_pos[ssort[gidx]]                       # [gt*P, K, 3]
            vals = np.where(mask[:, :, None], coords,
                            qp_l[jj][:, None, :]).astype(np.float16)
            v = vals.reshape(gt, P, K, 3).transpose(3, 1, 0, 2).reshape(3, P, gt * K)
            for m in range(3):
                planes[m].append(v[m])
            t0 += gt
        NBRp = [np.ascontiguousarray(np.concatenate(pl, axis=1)) for pl in planes]
        QPa = np.ascontiguousarray(
            qp_l.reshape(NT, P, 3).transpose(2, 1, 0).astype(np.float32))
        CNTTa = np.ascontiguousarray(deg_l.reshape(NT, P).astype(np.float32))
        in_maps.append({"NBRX": NBRp[0], "NBRY": NBRp[1], "NBRZ": NBRp[2],
                        "QP": QPa, "CNTT": CNTTa, "W1T": W1T, "B1C": B1C,
                        "W2TH": W2TH, "B2R": B2R})
    return in_maps, groups, perms


def kernel(source_pos, query_pos, edge_index, W1, b1, W2, b2):
    source_pos = np.asarray(source_pos, dtype=np.float32)
    query_pos = np.asarray(query_pos, dtype=np.float32)
    W1 = np.asarray(W1, dtype=np.float32)
    b1 = np.asarray(b1, dtype=np.float32)
    W2 = np.asarray(W2, dtype=np.float32)
    b2 = np.asarray(b2, dtype=np.float32)
    edge_index = np.asarray(edge_index)

    in_maps, groups, perms = _prep_inputs(source_pos, query_pos, edge_index,
                                          W1, b1, W2, b2)
    key = (NUM_CORES, groups)
    if key not in _BUILD_CACHE:
        _BUILD_CACHE[key] = build_module(NUM_CORES, groups)
    nc = _BUILD_CACHE[key]
    res = run_bass_kernel_spmd(nc, in_maps, core_ids=list(range(NUM_CORES)))
    out = np.empty((Q_NODES, OUT_DIM), dtype=np.float32)
    for c in range(NUM_CORES):
        arr = np.asarray(res.results[c]["OUT"])                    # [128, NQ_PAD]
        out[perms[c]] = arr[:, :NQ_CORE].T.astype(np.float32)
    return out
